# revision 1
# baseline (speedup 1.0000x reference)
"""Trainium2 Bass kernel for a 4-layer GCN (N=50000, D=128, E=1600000, 8 cores).

Strategy (graph/data parallel over destination nodes):
  - Nodes padded to 50176 = 392*128; each of 8 cores owns 6272 nodes (49 tiles).
  - Per layer, linearity lets us reorder:  out = dis * ((sum_{e->v} x'[src_e]) @ W) + b + prev
    with x' = dis * x  (self-loops appended as ordinary edges).
  - The scatter-sum runs as: dma_gather of x' rows (bf16, edge-major) +
    one-hot fp8 scatter matrices S streamed from DRAM, accumulated on the PE:
       psum_aggT[fi,dst] += msg_chunk[e,fi]^T @ S_chunk[e,dst]
  - aggT (SBUF, bf16) is then lhsT of a second matmul with W moving ->
    node-major psum_out[dst,fo]; epilogue (dis scale, +b+prev, relu) on DVE/ACT.
  - x'_next slices are AllGather'd into a fresh full table for the next layer.
  - int16 gather indices => table split in two 25088-row halves (A/B calls).
All preprocessing (degree, norm, edge partitioning, one-hot S, padding) is done
host-side in numpy; the edge structure is baked into the instruction stream.
"""

import sys

sys.path.insert(0, "/opt/trn_rl_repo")

import numpy as np
import ml_dtypes

N = 50000
D = 128
L = 4
E = 1600000
NCORES = 8
NPAD = 50176  # 392 * 128
NPC = NPAD // NCORES  # 6272 nodes per core
TPC = NPC // 128  # 49 tiles per core
HALF = NPAD // 2  # 25088, int16-indexable halves
GROUP = 4  # dst tiles per gather call pair
NGROUPS = (TPC + GROUP - 1) // GROUP  # 13

_compiled = None


def _preprocess(x, edge_index, W, b):
    src0 = edge_index[0].astype(np.int64)
    dst0 = edge_index[1].astype(np.int64)
    loops = np.arange(N, dtype=np.int64)
    src = np.concatenate([src0, loops])
    dst = np.concatenate([dst0, loops])
    deg = np.bincount(dst, minlength=N).astype(np.float32)
    dis = np.zeros(NPAD, np.float32)
    dis[:N] = 1.0 / np.sqrt(deg)

    # order edges by (core, local tile, half) once
    core_of = dst // NPC
    lt_of = (dst % NPC) // 128
    dloc_of = dst % 128
    half_of = (src >= HALF).astype(np.int64)
    key = ((core_of * TPC + lt_of) * 2 + half_of)
    order = np.argsort(key, kind="stable")
    src_s, key_s = src[order], key[order]
    dloc_s = dloc_of[order]
    # counts per (core, lt, half)
    counts = np.bincount(key_s, minlength=NCORES * TPC * 2).reshape(NCORES, TPC, 2)
    # shared chunk structure: per (lt, half) max over cores
    KCH = np.ceil(counts.max(axis=0) / 128.0).astype(np.int64)  # [TPC, 2]
    KCH = np.maximum(KCH, 1)

    # group layout (shared across cores): per group g covers tiles [g*GROUP, ...)
    # chunk stream order: g0(A tiles..., B tiles...), g1(...)
    tiles_of_group = [list(range(g * GROUP, min((g + 1) * GROUP, TPC))) for g in range(NGROUPS)]
    chA = [int(sum(KCH[t, 0] for t in tg)) for tg in tiles_of_group]
    chB = [int(sum(KCH[t, 1] for t in tg)) for tg in tiles_of_group]
    TOTCH = int(sum(chA) + sum(chB))

    # per-(lt,half) chunk offset in the global stream
    chunk_off = np.zeros((TPC, 2), np.int64)
    pos = 0
    for g, tg in enumerate(tiles_of_group):
        for h in (0, 1):
            for t in tg:
                chunk_off[t, h] = pos
                pos += KCH[t, h]
    assert pos == TOTCH

    # build per-core idx + S
    idx_all = np.zeros((NCORES, TOTCH * 128), np.int16)
    dlocs = np.full((NCORES, TOTCH * 128), -1, np.int16)
    starts = np.zeros(NCORES * TPC * 2 + 1, np.int64)
    np.cumsum(counts.reshape(-1), out=starts[1:])
    for c in range(NCORES):
        for t in range(TPC):
            for h in (0, 1):
                k = (c * TPC + t) * 2 + h
                s0, s1 = starts[k], starts[k + 1]
                n = s1 - s0
                o = chunk_off[t, h] * 128
                iv = src_s[s0:s1] - (HALF if h else 0)
                idx_all[c, o:o + n] = iv.astype(np.int16)
                dlocs[c, o:o + n] = dloc_s[s0:s1]

    # S matrices: [TOTCH, 128, 128] fp8, row e -> column dloc (skip -1)
    smat = np.zeros((NCORES, TOTCH, 128, 128), ml_dtypes.float8_e4m3)
    ii = np.arange(TOTCH * 128)
    for c in range(NCORES):
        d = dlocs[c]
        m = d >= 0
        smat[c][ii[m] // 128, ii[m] % 128, d[m]] = 1.0

    # idx tiles wrapped: [128, TOTCH*128//16], element i at [i%16, i//16], x8 groups
    idxw = np.zeros((NCORES, 128, TOTCH * 8), np.int16)
    for c in range(NCORES):
        w = idx_all[c].reshape(TOTCH * 8, 16).T
        for gme in range(8):
            idxw[c, gme * 16:(gme + 1) * 16, :] = w

    xpad = np.zeros((NPAD, D), np.float32)
    xpad[:N] = x
    x0p = (xpad * dis[:, None]).astype(ml_dtypes.bfloat16)
    disT = dis.reshape(NCORES, TPC, 128).transpose(0, 2, 1).copy()  # [c,128,TPC]
    b_bc = np.broadcast_to(b[None, :, :], (128, L, D)).astype(np.float32).copy()

    meta = dict(KCH=KCH, chA=chA, chB=chB, TOTCH=TOTCH, chunk_off=chunk_off,
                tiles_of_group=tiles_of_group)
    per_core = []
    for c in range(NCORES):
        per_core.append(dict(
            x_own=np.ascontiguousarray(xpad[c * NPC:(c + 1) * NPC]),
            x0p=x0p,
            w=W.astype(ml_dtypes.bfloat16),
            b_bc=b_bc,
            disT=np.ascontiguousarray(disT[c]),
            idxs=np.ascontiguousarray(idxw[c]),
            smat=np.ascontiguousarray(smat[c].reshape(TOTCH, 128, 128)),
        ))
    return meta, per_core


def _build(meta):
    from concourse import bacc, tile, bass_utils
    from concourse.bass import mybir

    KCH = meta["KCH"]
    chA, chB = meta["chA"], meta["chB"]
    TOTCH = meta["TOTCH"]
    chunk_off = meta["chunk_off"]
    tiles_of_group = meta["tiles_of_group"]
    MAXCH_G = max(chA[g] + chB[g] for g in range(NGROUPS))

    nc = bacc.Bacc("TRN2", target_bir_lowering=False, debug=False,
                   num_devices=NCORES)
    d_x_own = nc.dram_tensor("x_own", [NPC, D], mybir.dt.float32, kind="ExternalInput")
    d_x0p = nc.dram_tensor("x0p", [NPAD, D], mybir.dt.bfloat16, kind="ExternalInput")
    d_w = nc.dram_tensor("w", [L, D, D], mybir.dt.bfloat16, kind="ExternalInput")
    d_bbc = nc.dram_tensor("b_bc", [128, L, D], mybir.dt.float32, kind="ExternalInput")
    d_disT = nc.dram_tensor("disT", [128, TPC], mybir.dt.float32, kind="ExternalInput")
    d_idxs = nc.dram_tensor("idxs", [128, TOTCH * 8], mybir.dt.int16, kind="ExternalInput")
    d_smat = nc.dram_tensor("smat", [TOTCH, 128, 128], mybir.dt.float8e4, kind="ExternalInput")
    d_out = nc.dram_tensor("x_out", [NPC, D], mybir.dt.float32, kind="ExternalOutput")

    with tile.TileContext(nc) as tc:
        with (
            tc.tile_pool(name="const", bufs=1) as constp,
            tc.tile_pool(name="msg", bufs=2) as msgp,
            tc.tile_pool(name="sp", bufs=2) as sp_pool,
            tc.tile_pool(name="work", bufs=3) as workp,
            tc.tile_pool(name="xprime", bufs=3) as xpp,
            tc.tile_pool(name="pag", bufs=2, space="PSUM") as pagp,
            tc.tile_pool(name="pout", bufs=2, space="PSUM") as poutp,
            tc.tile_pool(name="dram", bufs=1, space="DRAM") as dramp,
        ):
            # ---- persistent SBUF state ----
            x_own = constp.tile([128, TPC, D], mybir.dt.float32, tag="x_own")
            nc.sync.dma_start(out=x_own[:], in_=d_x_own.ap().rearrange("(t p) f -> p t f", p=128))
            w_sb = constp.tile([128, L, D], mybir.dt.bfloat16, tag="w_sb")
            nc.sync.dma_start(out=w_sb[:], in_=d_w.ap().rearrange("l k f -> k l f"))
            bbc_sb = constp.tile([128, L, D], mybir.dt.float32, tag="bbc")
            nc.sync.dma_start(out=bbc_sb[:], in_=d_bbc.ap())
            disT_sb = constp.tile([128, TPC], mybir.dt.float32, tag="disT")
            nc.sync.dma_start(out=disT_sb[:], in_=d_disT.ap())
            idx_sb = constp.tile([128, TOTCH * 8], mybir.dt.int16, tag="idx")
            nc.sync.dma_start(out=idx_sb[:], in_=d_idxs.ap())

            # DRAM tables for layers 1..3 and AllGather input slices
            tables = [d_x0p.ap()]
            ag_ins = []
            for l in range(1, L):
                tab_tile = dramp.tile([NPAD, D], mybir.dt.bfloat16, tag=f"tab{l}")
                agin_tile = dramp.tile([NPC, D], mybir.dt.bfloat16, tag=f"agin{l}")
                tables.append(tab_tile[:])
                ag_ins.append(agin_tile[:])

            for l in range(L):
                table = tables[l]
                tabA = table[0:HALF, :]
                tabB = table[HALF:NPAD, :]
                for g in range(NGROUPS):
                    tg = tiles_of_group[g]
                    nA, nB = chA[g], chB[g]
                    goff = chunk_off[tg[0], 0]  # stream offset of this group
                    # S for the whole group in one DMA
                    s_t = sp_pool.tile([128, MAXCH_G, 128], mybir.dt.float8e4, tag="s_t")
                    nc.sync.dma_start(
                        out=s_t[:, 0:nA + nB, :],
                        in_=d_smat.ap()[goff:goff + nA + nB].rearrange("c p f -> p c f"),
                    )
                    # gather msg rows for both halves
                    msg = msgp.tile([128, MAXCH_G, D], mybir.dt.bfloat16, tag="msg")
                    if nA > 0:
                        nc.gpsimd.dma_gather(
                            out_ap=msg[:, 0:nA, :],
                            in_ap=tabA,
                            idxs_ap=idx_sb[:, goff * 8:(goff + nA) * 8],
                            num_idxs=nA * 128,
                            num_idxs_reg=nA * 128,
                            elem_size=D,
                            single_packet=False,
                        )
                    if nB > 0:
                        nc.gpsimd.dma_gather(
                            out_ap=msg[:, nA:nA + nB, :],
                            in_ap=tabB,
                            idxs_ap=idx_sb[:, (goff + nA) * 8:(goff + nA + nB) * 8],
                            num_idxs=nB * 128,
                            num_idxs_reg=nB * 128,
                            elem_size=D,
                            single_packet=False,
                        )
                    # per dst tile: accumulate one-hot matmuls, then @W + epilogue
                    for t in tg:
                        kA = int(KCH[t, 0])
                        kB = int(KCH[t, 1])
                        oA = int(chunk_off[t, 0] - goff)
                        oB = int(chunk_off[t, 1] - goff)
                        psA = pagp.tile([128, 128], mybir.dt.float32, tag="pag")
                        nchunks = kA + kB
                        ci = 0
                        for c in range(kA):
                            nc.tensor.matmul(psA[:], lhsT=msg[:, oA + c, :],
                                             rhs=s_t[:, oA + c, :],
                                             start=(ci == 0), stop=(ci == nchunks - 1))
                            ci += 1
                        for c in range(kB):
                            nc.tensor.matmul(psA[:], lhsT=msg[:, oB + c, :],
                                             rhs=s_t[:, oB + c, :],
                                             start=(ci == 0), stop=(ci == nchunks - 1))
                            ci += 1
                        aggT = workp.tile([128, 128], mybir.dt.bfloat16, tag="aggT")
                        nc.scalar.copy(aggT[:], psA[:])
                        pso = poutp.tile([128, 128], mybir.dt.float32, tag="pout")
                        nc.tensor.matmul(pso[:], lhsT=aggT[:], rhs=w_sb[:, l, :],
                                         start=True, stop=True)
                        # epilogue: xn = relu(dis*pso + b + prev); x' = bf16(dis*xn)
                        prevb = workp.tile([128, 128], mybir.dt.float32, tag="prevb")
                        nc.vector.tensor_tensor(
                            out=prevb[:], in0=x_own[:, t, :], in1=bbc_sb[:, l, :],
                            op=mybir.AluOpType.add)
                        t2 = workp.tile([128, 128], mybir.dt.float32, tag="t2")
                        nc.vector.tensor_scalar(
                            out=t2[:], in0=pso[:], scalar1=disT_sb[:, t:t + 1],
                            scalar2=None, op0=mybir.AluOpType.mult)
                        t3 = workp.tile([128, 128], mybir.dt.float32, tag="t3")
                        nc.vector.tensor_tensor(
                            out=t3[:], in0=t2[:], in1=prevb[:], op=mybir.AluOpType.add)
                        nc.scalar.activation(
                            out=x_own[:, t, :], in_=t3[:],
                            func=mybir.ActivationFunctionType.Relu)
                        if l < L - 1:
                            xpr = xpp.tile([128, 128], mybir.dt.bfloat16, tag="xpr")
                            nc.scalar.activation(
                                out=xpr[:], in_=x_own[:, t, :],
                                func=mybir.ActivationFunctionType.Copy,
                                scale=disT_sb[:, t:t + 1])
                            nc.sync.dma_start(
                                out=ag_ins[l].rearrange("(t p) f -> p t f", p=128)[:, t, :],
                                in_=xpr[:])
                if l < L - 1:
                    nc.gpsimd.collective_compute(
                        "AllGather",
                        mybir.AluOpType.bypass,
                        replica_groups=[list(range(NCORES))],
                        ins=[ag_ins[l].opt()],
                        outs=[tables[l + 1].opt()],
                    )
            nc.sync.dma_start(out=d_out.ap().rearrange("(t p) f -> p t f", p=128),
                              in_=x_own[:])

    nc.compile()
    return nc


def kernel(x, edge_index, W, b):
    global _compiled
    from concourse import bass_utils

    x = np.asarray(x, dtype=np.float32)
    W_np = np.asarray(W, dtype=np.float32)
    b_np = np.asarray(b, dtype=np.float32)
    ei = np.asarray(edge_index)

    meta, per_core = _preprocess(x, ei, W_np, b_np)
    globals()["_last_per_core"] = per_core
    if _compiled is None:
        _compiled = _build(meta)
    nc = _compiled
    res = bass_utils.run_bass_kernel_spmd(nc, per_core, core_ids=list(range(NCORES)))
    out = np.concatenate([res.results[c]["x_out"] for c in range(NCORES)], axis=0)
    return out[:N].astype(np.float32)



# revision 4
# speedup vs baseline: 1.8428x; 1.8428x over previous
"""Trainium2 Bass kernel for a 4-layer GCN (N=50000, D=128, E=1600000, 8 cores).

Hybrid aggregation (v2):
  - Nodes padded to 50176 = 392*128; each of 8 cores owns 6272 nodes (49 tiles).
  - Per layer, linearity lets us reorder: out = dis * ((sum_{e->v} x'[src_e]) @ W) + b + prev
    with x' = dis * x.
  - Source tiles split: DENSE = global tiles [0, 200) (same for all cores, so
    the SPMD instruction stream is identical; per-core data differs), GATHER =
    tiles [200, 392). Self-loops are added via an identity matmul from the
    SBUF-resident x' own slice.
  - DENSE path: host-built fp8 count matrices B[u, dst_local] stream from HBM;
    PE accumulates psum_aggT[fi,dst] += x'_s^T @ B_s (N=512 moving operands).
    No per-edge work on any engine.
  - GATHER path: dma_gather of x' rows (bf16, edge-major; ~8ns/row Q7
    emission) + one-hot fp8 S matrices streamed from DRAM:
    psum_aggT[fi,dst] += msg_chunk[e,fi]^T @ S_chunk[e,dst].
    Gathered srcs are all >= 25600, span 24576 < 2^15 -> single int16 range.
  - 49 dst tiles -> 13 groups of <=4; pass A = groups 0-5, pass B = 6-12 so
    live [128,512] fp32 PSUM banks stay <= 7 (+1 for the W matmul).
  - aggT (bf16) is lhsT of the W matmul -> psum_out[dst,fo]; epilogue
    (dis scale, +b+prev, relu) on DVE/ACT; x' slices AllGather'd into the
    next layer's table.
"""

import sys

sys.path.insert(0, "/opt/trn_rl_repo")

import numpy as np
import ml_dtypes

N = 50000
D = 128
L = 4
E = 1600000
NCORES = 8
NPAD = 50176        # 392 * 128
NT = NPAD // 128    # 392 src tiles
NPC = NPAD // NCORES  # 6272
TPC = NPC // 128    # 49
M_DENSE = 200       # dense src tiles (global ids [0, M_DENSE))
DENSE_ROWS = M_DENSE * 128  # 25600
DBLK = 8            # dense tiles per DMA block
NBLK = M_DENSE // DBLK  # 25
GROUP = 4
NGROUPS = (TPC + GROUP - 1) // GROUP  # 13 (last group = 1 tile)
WA = 24 * 128       # pass A dst width (tiles 0..23)
WB = 25 * 128       # pass B dst width (tiles 24..48)
PASS_GROUPS = [list(range(0, 6)), list(range(6, 13))]

_compiled = None


def _preprocess(x, edge_index, W, b):
    fp8 = ml_dtypes.float8_e4m3
    src0 = edge_index[0].astype(np.int64)
    dst0 = edge_index[1].astype(np.int64)
    loops = np.arange(N, dtype=np.int64)
    # self-loops handled separately on device (identity matmul)
    deg = np.bincount(np.concatenate([dst0, loops]), minlength=N).astype(np.float32)
    dis = np.zeros(NPAD, np.float32)
    dis[:N] = 1.0 / np.sqrt(deg)

    core_of = dst0 // NPC
    is_dense = src0 < DENSE_ROWS

    # ---------------- dense path: B count blocks ----------------
    BA = np.zeros((NCORES, NBLK, 128, DBLK, WA), np.uint8)
    BB = np.zeros((NCORES, NBLK, 128, DBLK, WB), np.uint8)
    de = np.where(is_dense)[0]
    d_core = core_of[de]
    st = src0[de] // 128
    d_blk = st // DBLK
    d_k = st % DBLK
    d_u = src0[de] % 128
    d_dloc = dst0[de] - d_core * NPC
    in_a = d_dloc < WA
    np.add.at(BA, (d_core[in_a], d_blk[in_a], d_u[in_a], d_k[in_a], d_dloc[in_a]), 1)
    ib = ~in_a
    np.add.at(BB, (d_core[ib], d_blk[ib], d_u[ib], d_k[ib], d_dloc[ib] - WA), 1)

    # ---------------- gather path ----------------
    ge = np.where(~is_dense)[0]
    gsrc, gdst = src0[ge], dst0[ge]
    g_core = core_of[ge]
    lt_of = (gdst % NPC) // 128
    dloc_of = gdst % 128
    key = g_core * TPC + lt_of
    order = np.argsort(key, kind="stable")
    src_s, key_s = gsrc[order], key[order]
    dloc_s = dloc_of[order]
    counts = np.bincount(key_s, minlength=NCORES * TPC).reshape(NCORES, TPC)
    KCH = np.maximum(np.ceil(counts.max(axis=0) / 128.0).astype(np.int64), 1)  # [TPC]

    tiles_of_group = [list(range(g * GROUP, min((g + 1) * GROUP, TPC)))
                      for g in range(NGROUPS)]
    chG = [int(sum(KCH[t] for t in tg)) for tg in tiles_of_group]
    TOTCH = int(sum(chG))

    chunk_off = np.zeros(TPC, np.int64)
    pos = 0
    for tg in tiles_of_group:
        for t in tg:
            chunk_off[t] = pos
            pos += KCH[t]
    assert pos == TOTCH

    idx_all = np.zeros((NCORES, TOTCH * 128), np.int16)
    dlocs = np.full((NCORES, TOTCH * 128), -1, np.int16)
    starts = np.zeros(NCORES * TPC + 1, np.int64)
    np.cumsum(counts.reshape(-1), out=starts[1:])
    for c in range(NCORES):
        for t in range(TPC):
            k = c * TPC + t
            s0, s1 = starts[k], starts[k + 1]
            n = s1 - s0
            o = chunk_off[t] * 128
            idx_all[c, o:o + n] = (src_s[s0:s1] - DENSE_ROWS).astype(np.int16)
            dlocs[c, o:o + n] = dloc_s[s0:s1]

    smat = np.zeros((NCORES, TOTCH, 128, 128), fp8)
    ii = np.arange(TOTCH * 128)
    for c in range(NCORES):
        d = dlocs[c]
        m = d >= 0
        smat[c][ii[m] // 128, ii[m] % 128, d[m]] = 1.0

    idxw = np.zeros((NCORES, 128, TOTCH * 8), np.int16)
    for c in range(NCORES):
        w = idx_all[c].reshape(TOTCH * 8, 16).T
        for gme in range(8):
            idxw[c, gme * 16:(gme + 1) * 16, :] = w

    xpad = np.zeros((NPAD, D), np.float32)
    xpad[:N] = x
    x0p = (xpad * dis[:, None]).astype(ml_dtypes.bfloat16)
    disT = dis.reshape(NCORES, TPC, 128).transpose(0, 2, 1).copy()  # [c,128,TPC]
    b_bc = np.broadcast_to(b[None, :, :], (128, L, D)).astype(np.float32).copy()
    ident = np.eye(128, dtype=np.float32).astype(ml_dtypes.bfloat16)

    meta = dict(KCH=KCH, chG=chG, TOTCH=TOTCH, chunk_off=chunk_off,
                tiles_of_group=tiles_of_group)
    per_core = []
    for c in range(NCORES):
        xo = xpad[c * NPC:(c + 1) * NPC]
        x0p_own = np.ascontiguousarray(
            x0p[c * NPC:(c + 1) * NPC].reshape(TPC, 128, D).transpose(1, 0, 2))
        per_core.append(dict(
            x_own=np.ascontiguousarray(xo),
            x0p=x0p,
            x0p_own=x0p_own,
            w=W.astype(ml_dtypes.bfloat16),
            b_bc=b_bc,
            disT=np.ascontiguousarray(disT[c]),
            ident=ident,
            idxs=np.ascontiguousarray(idxw[c]),
            smat=np.ascontiguousarray(smat[c].reshape(TOTCH, 128, 128)),
            ba=BA[c].astype(fp8),
            bb=BB[c].astype(fp8),
        ))
    return meta, per_core


def _build(meta):
    from concourse import bacc, tile
    from concourse.bass import mybir

    KCH = meta["KCH"]
    chG = meta["chG"]
    TOTCH = meta["TOTCH"]
    chunk_off = meta["chunk_off"]
    tiles_of_group = meta["tiles_of_group"]
    MAXCH_G = max(chG)

    nc = bacc.Bacc("TRN2", target_bir_lowering=False, debug=False,
                   num_devices=NCORES)
    d_x_own = nc.dram_tensor("x_own", [NPC, D], mybir.dt.float32, kind="ExternalInput")
    d_x0p = nc.dram_tensor("x0p", [NPAD, D], mybir.dt.bfloat16, kind="ExternalInput")
    d_x0po = nc.dram_tensor("x0p_own", [128, TPC, D], mybir.dt.bfloat16, kind="ExternalInput")
    d_w = nc.dram_tensor("w", [L, D, D], mybir.dt.bfloat16, kind="ExternalInput")
    d_bbc = nc.dram_tensor("b_bc", [128, L, D], mybir.dt.float32, kind="ExternalInput")
    d_disT = nc.dram_tensor("disT", [128, TPC], mybir.dt.float32, kind="ExternalInput")
    d_ident = nc.dram_tensor("ident", [128, 128], mybir.dt.bfloat16, kind="ExternalInput")
    d_idxs = nc.dram_tensor("idxs", [128, TOTCH * 8], mybir.dt.int16, kind="ExternalInput")
    d_smat = nc.dram_tensor("smat", [TOTCH, 128, 128], mybir.dt.float8e4, kind="ExternalInput")
    d_ba = nc.dram_tensor("ba", [NBLK, 128, DBLK, WA], mybir.dt.float8e4, kind="ExternalInput")
    d_bb = nc.dram_tensor("bb", [NBLK, 128, DBLK, WB], mybir.dt.float8e4, kind="ExternalInput")
    d_out = nc.dram_tensor("x_out", [NPC, D], mybir.dt.float32, kind="ExternalOutput")

    with tile.TileContext(nc) as tc:
        with (
            tc.tile_pool(name="const", bufs=1) as constp,
            tc.tile_pool(name="msg", bufs=2) as msgp,
            tc.tile_pool(name="sp", bufs=2) as sp_pool,
            tc.tile_pool(name="xd", bufs=3) as xdp,
            tc.tile_pool(name="bblk", bufs=2) as bbp,
            tc.tile_pool(name="work", bufs=3) as workp,
            tc.tile_pool(name="aggt", bufs=2) as aggp,
            tc.tile_pool(name="pag", bufs=7, space="PSUM") as pagp,
            tc.tile_pool(name="pout", bufs=1, space="PSUM") as poutp,
            tc.tile_pool(name="dram", bufs=1, space="DRAM") as dramp,
        ):
            # ---- persistent SBUF state ----
            x_own = constp.tile([128, TPC, D], mybir.dt.float32, tag="x_own")
            nc.sync.dma_start(out=x_own[:], in_=d_x_own.ap().rearrange("(t p) f -> p t f", p=128))
            xpr_all = constp.tile([128, TPC, D], mybir.dt.bfloat16, tag="xpr_all")
            nc.sync.dma_start(out=xpr_all[:], in_=d_x0po.ap())
            w_sb = constp.tile([128, L, D], mybir.dt.bfloat16, tag="w_sb")
            nc.sync.dma_start(out=w_sb[:], in_=d_w.ap().rearrange("l k f -> k l f"))
            bbc_sb = constp.tile([128, L, D], mybir.dt.float32, tag="bbc")
            nc.sync.dma_start(out=bbc_sb[:], in_=d_bbc.ap())
            disT_sb = constp.tile([128, TPC], mybir.dt.float32, tag="disT")
            nc.sync.dma_start(out=disT_sb[:], in_=d_disT.ap())
            ident_sb = constp.tile([128, 128], mybir.dt.bfloat16, tag="ident")
            nc.sync.dma_start(out=ident_sb[:], in_=d_ident.ap())
            idx_sb = constp.tile([128, TOTCH * 8], mybir.dt.int16, tag="idx")
            nc.sync.dma_start(out=idx_sb[:], in_=d_idxs.ap())

            # DRAM tables for layers 1..3 and AllGather input slices
            tables = [d_x0p.ap()]
            ag_ins = []
            for l in range(1, L):
                tab_tile = dramp.tile([NPAD, D], mybir.dt.bfloat16, tag=f"tab{l}")
                agin_tile = dramp.tile([NPC, D], mybir.dt.bfloat16, tag=f"agin{l}")
                tables.append(tab_tile[:])
                ag_ins.append(agin_tile[:])

            for l in range(L):
                table = tables[l]
                tabG = table[DENSE_ROWS:NPAD, :]
                for pi, groups in enumerate(PASS_GROUPS):
                    d_b = d_ba if pi == 0 else d_bb
                    wpass = WA if pi == 0 else WB
                    base = 0 if pi == 0 else WA
                    psums = {}
                    for g in groups:
                        pgt = pagp.tile([128, 512], mybir.dt.float32, tag="pag",
                                        name=f"pag_{l}_{g}")
                        psums[g] = pgt
                    # ---- dense stream ----
                    for blk in range(NBLK):
                        xd = xdp.tile([128, DBLK, D], mybir.dt.bfloat16, tag="xd")
                        nc.sync.dma_start(
                            out=xd[:],
                            in_=table[blk * DBLK * 128:(blk + 1) * DBLK * 128, :]
                                .rearrange("(k p) f -> p k f", p=128))
                        bbl = bbp.tile([128, DBLK, wpass], mybir.dt.float8e4, tag="bblk")
                        nc.sync.dma_start(out=bbl[:], in_=d_b.ap()[blk])
                        for k in range(DBLK):
                            for gi, g in enumerate(groups):
                                goff = gi * 512
                                wid = min(512, wpass - goff)
                                nc.tensor.matmul(
                                    psums[g][:, 0:wid],
                                    lhsT=xd[:, k, :],
                                    rhs=bbl[:, k, goff:goff + wid],
                                    start=(blk == 0 and k == 0), stop=False)
                    # ---- gather stream ----
                    for g in groups:
                        tg = tiles_of_group[g]
                        nch = chG[g]
                        goff = int(chunk_off[tg[0]])
                        s_t = sp_pool.tile([128, MAXCH_G, 128], mybir.dt.float8e4, tag="s_t")
                        nc.sync.dma_start(
                            out=s_t[:, 0:nch, :],
                            in_=d_smat.ap()[goff:goff + nch].rearrange("c p f -> p c f"))
                        msg = msgp.tile([128, MAXCH_G, D], mybir.dt.bfloat16, tag="msg")
                        nc.gpsimd.dma_gather(
                            out_ap=msg[:, 0:nch, :],
                            in_ap=tabG,
                            idxs_ap=idx_sb[:, goff * 8:(goff + nch) * 8],
                            num_idxs=nch * 128,
                            num_idxs_reg=nch * 128,
                            elem_size=D,
                            single_packet=False,
                        )
                        for tl, t in enumerate(tg):
                            oT = int(chunk_off[t] - goff)
                            for c in range(int(KCH[t])):
                                nc.tensor.matmul(
                                    psums[g][:, tl * 128:(tl + 1) * 128],
                                    lhsT=msg[:, oT + c, :],
                                    rhs=s_t[:, oT + c, :],
                                    start=False, stop=False)
                    # ---- self-loops, close, epilogue ----
                    for g in groups:
                        tg = tiles_of_group[g]
                        for tl, t in enumerate(tg):
                            nc.tensor.matmul(
                                psums[g][:, tl * 128:(tl + 1) * 128],
                                lhsT=xpr_all[:, t, :],
                                rhs=ident_sb[:],
                                start=False, stop=(tl == len(tg) - 1))
                        wid = len(tg) * 128
                        aggT = aggp.tile([128, 512], mybir.dt.bfloat16, tag="aggT")
                        nc.scalar.copy(aggT[:, 0:wid], psums[g][:, 0:wid])
                        for tl, t in enumerate(tg):
                            pso = poutp.tile([128, 128], mybir.dt.float32, tag="pout")
                            nc.tensor.matmul(pso[:], lhsT=aggT[:, tl * 128:(tl + 1) * 128],
                                             rhs=w_sb[:, l, :], start=True, stop=True)
                            prevb = workp.tile([128, 128], mybir.dt.float32, tag="prevb")
                            nc.vector.tensor_tensor(
                                out=prevb[:], in0=x_own[:, t, :], in1=bbc_sb[:, l, :],
                                op=mybir.AluOpType.add)
                            t2 = workp.tile([128, 128], mybir.dt.float32, tag="t2")
                            nc.vector.tensor_scalar(
                                out=t2[:], in0=pso[:], scalar1=disT_sb[:, t:t + 1],
                                scalar2=None, op0=mybir.AluOpType.mult)
                            t3 = workp.tile([128, 128], mybir.dt.float32, tag="t3")
                            nc.vector.tensor_tensor(
                                out=t3[:], in0=t2[:], in1=prevb[:], op=mybir.AluOpType.add)
                            nc.scalar.activation(
                                out=x_own[:, t, :], in_=t3[:],
                                func=mybir.ActivationFunctionType.Relu)
                            if l < L - 1:
                                nc.scalar.activation(
                                    out=xpr_all[:, t, :], in_=x_own[:, t, :],
                                    func=mybir.ActivationFunctionType.Copy,
                                    scale=disT_sb[:, t:t + 1])
                        if l < L - 1:
                            t0, t1 = tg[0], tg[-1] + 1
                            nc.sync.dma_start(
                                out=ag_ins[l].rearrange("(t p) f -> p t f", p=128)[:, t0:t1, :],
                                in_=xpr_all[:, t0:t1, :])
                if l < L - 1:
                    nc.gpsimd.collective_compute(
                        "AllGather",
                        mybir.AluOpType.bypass,
                        replica_groups=[list(range(NCORES))],
                        ins=[ag_ins[l].opt()],
                        outs=[tables[l + 1].opt()],
                    )
            nc.sync.dma_start(out=d_out.ap().rearrange("(t p) f -> p t f", p=128),
                              in_=x_own[:])

    nc.compile()
    return nc


def kernel(x, edge_index, W, b):
    global _compiled
    from concourse import bass_utils

    x = np.asarray(x, dtype=np.float32)
    W_np = np.asarray(W, dtype=np.float32)
    b_np = np.asarray(b, dtype=np.float32)
    ei = np.asarray(edge_index)

    meta, per_core = _preprocess(x, ei, W_np, b_np)
    globals()["_last_per_core"] = per_core
    if _compiled is None:
        _compiled = _build(meta)
    nc = _compiled
    res = bass_utils.run_bass_kernel_spmd(nc, per_core, core_ids=list(range(NCORES)))
    out = np.concatenate([res.results[c]["x_out"] for c in range(NCORES)], axis=0)
    return out[:N].astype(np.float32)


# revision 5
# speedup vs baseline: 2.4351x; 1.3214x over previous
"""Trainium2 Bass kernel for a 4-layer GCN (N=50000, D=128, E=1600000, 8 cores).

Hybrid aggregation (v3, group-major pipeline):
  - Nodes padded to 50176 = 392*128; each core owns 6272 nodes (49 tiles).
  - out = dis * ((sum_{e->v} x'[src_e]) @ W) + b + prev, x' = dis * x.
  - Src tiles split: DENSE = global tiles [0, M_DENSE) (same for all cores so
    the SPMD stream is identical; per-core B data differs), GATHER = the rest.
    Self-loops via an identity matmul from the SBUF-resident x' own slice.
  - 49 dst tiles -> 13 groups of <=4 (512 dst columns, one PSUM bank each),
    processed as a pipeline; per group:
      dense:  psum[fi,dst] += xall[:,s,:]^T @ B[s, g-slice]   (fp8 B streamed
              from HBM in 24-src-tile sub-blocks, N=512 moving operands)
      gather: dma_gather x' rows (bf16 edge-major, ~8ns/row Q7) + one-hot fp8
              S chunks: psum += msg_chunk^T @ S_chunk  (per 2-tile units)
      self:   psum[:, tl] += xpr_own_tile^T @ I
      close:  aggT bf16 <- psum; per tile: @W -> psum_out; epilogue on DVE/ACT
  - x' slices AllGather'd into the next layer's table; dense x' tiles bulk
    reloaded to SBUF each layer (no per-edge descriptors on the dense path).
"""

import sys

sys.path.insert(0, "/opt/trn_rl_repo")

import numpy as np
import ml_dtypes

N = 50000
D = 128
L = 4
E = 1600000
NCORES = 8
NPAD = 50176          # 392 * 128
NT = NPAD // 128      # 392 src tiles
NPC = NPAD // NCORES  # 6272
TPC = NPC // 128      # 49
M_DENSE = 192         # dense src tiles (global ids [0, M_DENSE))
DENSE_ROWS = M_DENSE * 128  # 24576; gathered span 25600 < 2^15
SBW = 24              # dense src tiles per B sub-block DMA
NSB = M_DENSE // SBW  # 8
GROUP = 4
NGROUPS = (TPC + GROUP - 1) // GROUP  # 13 (last group = 1 tile)
GW = 512              # psum group width (4*128)

_compiled = None


def _units_of_group(tg):
    return [tg[i:i + 2] for i in range(0, len(tg), 2)]


def _preprocess(x, edge_index, W, b):
    fp8 = ml_dtypes.float8_e4m3
    src0 = edge_index[0].astype(np.int64)
    dst0 = edge_index[1].astype(np.int64)
    loops = np.arange(N, dtype=np.int64)
    deg = np.bincount(np.concatenate([dst0, loops]), minlength=N).astype(np.float32)
    dis = np.zeros(NPAD, np.float32)
    dis[:N] = 1.0 / np.sqrt(deg)

    core_of = dst0 // NPC
    is_dense = src0 < DENSE_ROWS

    # ---------------- dense path: B count blocks, group-major ----------------
    # BG[c] layout: [NGROUPS, NSB, 128(u), SBW(k), GW(dst col)]
    BG = np.zeros((NCORES, NGROUPS, NSB, 128, SBW, GW), np.uint8)
    de = np.where(is_dense)[0]
    d_core = core_of[de]
    st = src0[de] // 128
    d_sb = st // SBW
    d_k = st % SBW
    d_u = src0[de] % 128
    d_dloc = dst0[de] - d_core * NPC
    d_g = d_dloc // GW
    d_col = d_dloc % GW
    np.add.at(BG, (d_core, d_g, d_sb, d_u, d_k, d_col), 1)

    # ---------------- gather path ----------------
    ge = np.where(~is_dense)[0]
    gsrc, gdst = src0[ge], dst0[ge]
    g_core = core_of[ge]
    lt_of = (gdst % NPC) // 128
    dloc_of = gdst % 128
    key = g_core * TPC + lt_of
    order = np.argsort(key, kind="stable")
    src_s = gsrc[order]
    dloc_s = dloc_of[order]
    counts = np.bincount(key, minlength=NCORES * TPC).reshape(NCORES, TPC)
    KCH = np.maximum(np.ceil(counts.max(axis=0) / 128.0).astype(np.int64), 1)  # [TPC]

    tiles_of_group = [list(range(g * GROUP, min((g + 1) * GROUP, TPC)))
                      for g in range(NGROUPS)]
    TOTCH = int(KCH.sum())
    chunk_off = np.zeros(TPC, np.int64)
    pos = 0
    for tg in tiles_of_group:
        for t in tg:
            chunk_off[t] = pos
            pos += KCH[t]
    assert pos == TOTCH

    idx_all = np.zeros((NCORES, TOTCH * 128), np.int16)
    dlocs = np.full((NCORES, TOTCH * 128), -1, np.int16)
    starts = np.zeros(NCORES * TPC + 1, np.int64)
    np.cumsum(counts.reshape(-1), out=starts[1:])
    for c in range(NCORES):
        for t in range(TPC):
            k = c * TPC + t
            s0, s1 = starts[k], starts[k + 1]
            n = s1 - s0
            o = chunk_off[t] * 128
            idx_all[c, o:o + n] = (src_s[s0:s1] - DENSE_ROWS).astype(np.int16)
            dlocs[c, o:o + n] = dloc_s[s0:s1]

    smat = np.zeros((NCORES, TOTCH, 128, 128), fp8)
    ii = np.arange(TOTCH * 128)
    for c in range(NCORES):
        d = dlocs[c]
        m = d >= 0
        smat[c][ii[m] // 128, ii[m] % 128, d[m]] = 1.0

    idxw = np.zeros((NCORES, 128, TOTCH * 8), np.int16)
    for c in range(NCORES):
        w = idx_all[c].reshape(TOTCH * 8, 16).T
        for gme in range(8):
            idxw[c, gme * 16:(gme + 1) * 16, :] = w

    xpad = np.zeros((NPAD, D), np.float32)
    xpad[:N] = x
    x0p = (xpad * dis[:, None]).astype(ml_dtypes.bfloat16)
    disT = dis.reshape(NCORES, TPC, 128).transpose(0, 2, 1).copy()  # [c,128,TPC]
    b_bc = np.broadcast_to(b[None, :, :], (128, L, D)).astype(np.float32).copy()
    ident = np.eye(128, dtype=np.float32).astype(ml_dtypes.bfloat16)

    meta = dict(KCH=KCH, TOTCH=TOTCH, chunk_off=chunk_off,
                tiles_of_group=tiles_of_group)
    per_core = []
    for c in range(NCORES):
        x0p_own = np.ascontiguousarray(
            x0p[c * NPC:(c + 1) * NPC].reshape(TPC, 128, D).transpose(1, 0, 2))
        per_core.append(dict(
            x_own=np.ascontiguousarray(xpad[c * NPC:(c + 1) * NPC]),
            x0p=x0p,
            x0p_own=x0p_own,
            w=W.astype(ml_dtypes.bfloat16),
            b_bc=b_bc,
            disT=np.ascontiguousarray(disT[c]),
            ident=ident,
            idxs=np.ascontiguousarray(idxw[c]),
            smat=np.ascontiguousarray(smat[c].reshape(TOTCH, 128, 128)),
            bg=BG[c].astype(fp8),
        ))
    return meta, per_core


def _build(meta):
    from concourse import bacc, tile
    from concourse.bass import mybir

    KCH = meta["KCH"]
    TOTCH = meta["TOTCH"]
    chunk_off = meta["chunk_off"]
    tiles_of_group = meta["tiles_of_group"]
    MAXCH_U = max(int(sum(KCH[t] for t in u))
                  for tg in tiles_of_group for u in _units_of_group(tg))

    nc = bacc.Bacc("TRN2", target_bir_lowering=False, debug=False,
                   num_devices=NCORES)
    d_x_own = nc.dram_tensor("x_own", [NPC, D], mybir.dt.float32, kind="ExternalInput")
    d_x0p = nc.dram_tensor("x0p", [NPAD, D], mybir.dt.bfloat16, kind="ExternalInput")
    d_x0po = nc.dram_tensor("x0p_own", [128, TPC, D], mybir.dt.bfloat16, kind="ExternalInput")
    d_w = nc.dram_tensor("w", [L, D, D], mybir.dt.bfloat16, kind="ExternalInput")
    d_bbc = nc.dram_tensor("b_bc", [128, L, D], mybir.dt.float32, kind="ExternalInput")
    d_disT = nc.dram_tensor("disT", [128, TPC], mybir.dt.float32, kind="ExternalInput")
    d_ident = nc.dram_tensor("ident", [128, 128], mybir.dt.bfloat16, kind="ExternalInput")
    d_idxs = nc.dram_tensor("idxs", [128, TOTCH * 8], mybir.dt.int16, kind="ExternalInput")
    d_smat = nc.dram_tensor("smat", [TOTCH, 128, 128], mybir.dt.float8e4, kind="ExternalInput")
    d_bg = nc.dram_tensor("bg", [NGROUPS, NSB, 128, SBW, GW], mybir.dt.float8e4,
                          kind="ExternalInput")
    d_out = nc.dram_tensor("x_out", [NPC, D], mybir.dt.float32, kind="ExternalOutput")

    with tile.TileContext(nc) as tc:
        with (
            tc.tile_pool(name="const", bufs=1) as constp,
            tc.tile_pool(name="xallp", bufs=1) as xallp,
            tc.tile_pool(name="msg", bufs=3) as msgp,
            tc.tile_pool(name="sp", bufs=3) as sp_pool,
            tc.tile_pool(name="bblk", bufs=2) as bbp,
            tc.tile_pool(name="work", bufs=3) as workp,
            tc.tile_pool(name="aggt", bufs=2) as aggp,
            tc.tile_pool(name="pag", bufs=3, space="PSUM") as pagp,
            tc.tile_pool(name="pout", bufs=2, space="PSUM") as poutp,
            tc.tile_pool(name="dram", bufs=1, space="DRAM") as dramp,
        ):
            x_own = constp.tile([128, TPC, D], mybir.dt.float32, tag="x_own")
            nc.sync.dma_start(out=x_own[:], in_=d_x_own.ap().rearrange("(t p) f -> p t f", p=128))
            xpr_all = constp.tile([128, TPC, D], mybir.dt.bfloat16, tag="xpr_all")
            nc.sync.dma_start(out=xpr_all[:], in_=d_x0po.ap())
            w_sb = constp.tile([128, L, D], mybir.dt.bfloat16, tag="w_sb")
            nc.sync.dma_start(out=w_sb[:], in_=d_w.ap().rearrange("l k f -> k l f"))
            bbc_sb = constp.tile([128, L, D], mybir.dt.float32, tag="bbc")
            nc.sync.dma_start(out=bbc_sb[:], in_=d_bbc.ap())
            disT_sb = constp.tile([128, TPC], mybir.dt.float32, tag="disT")
            nc.sync.dma_start(out=disT_sb[:], in_=d_disT.ap())
            ident_sb = constp.tile([128, 128], mybir.dt.bfloat16, tag="ident")
            nc.sync.dma_start(out=ident_sb[:], in_=d_ident.ap())
            idx_sb = constp.tile([128, TOTCH * 8], mybir.dt.int16, tag="idx")
            nc.sync.dma_start(out=idx_sb[:], in_=d_idxs.ap())

            tables = [d_x0p.ap()]
            ag_ins = []
            for l in range(1, L):
                tab_tile = dramp.tile([NPAD, D], mybir.dt.bfloat16, tag=f"tab{l}")
                agin_tile = dramp.tile([NPC, D], mybir.dt.bfloat16, tag=f"agin{l}")
                tables.append(tab_tile[:])
                ag_ins.append(agin_tile[:])

            for l in range(L):
                table = tables[l]
                tabG = table[DENSE_ROWS:NPAD, :]
                xall = xallp.tile([128, M_DENSE, D], mybir.dt.bfloat16,
                                  tag="xall", name=f"xall_{l}")
                nc.sync.dma_start(
                    out=xall[:],
                    in_=table[0:DENSE_ROWS, :].rearrange("(t p) f -> p t f", p=128))
                for g in range(NGROUPS):
                    tg = tiles_of_group[g]
                    psum_g = pagp.tile([128, GW], mybir.dt.float32, tag="pag",
                                       name=f"pag_{l}_{g}")
                    # ---- dense stream ----
                    for sb in range(NSB):
                        bsub = bbp.tile([128, SBW, GW], mybir.dt.float8e4, tag="bsub",
                                        name=f"bsub_{l}_{g}_{sb}")
                        nc.sync.dma_start(out=bsub[:], in_=d_bg.ap()[g, sb])
                        for k in range(SBW):
                            nc.tensor.matmul(
                                psum_g[:],
                                lhsT=xall[:, sb * SBW + k, :],
                                rhs=bsub[:, k, :],
                                start=(sb == 0 and k == 0), stop=False)
                    # ---- gather stream (2-tile units) ----
                    for u in _units_of_group(tg):
                        nch = int(sum(KCH[t] for t in u))
                        goff = int(chunk_off[u[0]])
                        s_t = sp_pool.tile([128, MAXCH_U, 128], mybir.dt.float8e4,
                                           tag="s_t", name=f"s_t_{l}_{g}_{u[0]}")
                        nc.sync.dma_start(
                            out=s_t[:, 0:nch, :],
                            in_=d_smat.ap()[goff:goff + nch].rearrange("c p f -> p c f"))
                        msg = msgp.tile([128, MAXCH_U, D], mybir.dt.bfloat16,
                                        tag="msg", name=f"msg_{l}_{g}_{u[0]}")
                        nc.gpsimd.dma_gather(
                            out_ap=msg[:, 0:nch, :],
                            in_ap=tabG,
                            idxs_ap=idx_sb[:, goff * 8:(goff + nch) * 8],
                            num_idxs=nch * 128,
                            num_idxs_reg=nch * 128,
                            elem_size=D,
                            single_packet=False,
                        )
                        for t in u:
                            tl = t - tg[0]
                            oT = int(chunk_off[t] - goff)
                            for c in range(int(KCH[t])):
                                nc.tensor.matmul(
                                    psum_g[:, tl * 128:(tl + 1) * 128],
                                    lhsT=msg[:, oT + c, :],
                                    rhs=s_t[:, oT + c, :],
                                    start=False, stop=False)
                    # ---- self-loops, close, epilogue ----
                    for tl, t in enumerate(tg):
                        nc.tensor.matmul(
                            psum_g[:, tl * 128:(tl + 1) * 128],
                            lhsT=xpr_all[:, t, :],
                            rhs=ident_sb[:],
                            start=False, stop=(tl == len(tg) - 1))
                    wid = len(tg) * 128
                    aggT = aggp.tile([128, GW], mybir.dt.bfloat16, tag="aggT",
                                     name=f"aggT_{l}_{g}")
                    nc.scalar.copy(aggT[:, 0:wid], psum_g[:, 0:wid])
                    for tl, t in enumerate(tg):
                        pso = poutp.tile([128, 128], mybir.dt.float32, tag="pout",
                                         name=f"pout_{l}_{t}")
                        nc.tensor.matmul(pso[:], lhsT=aggT[:, tl * 128:(tl + 1) * 128],
                                         rhs=w_sb[:, l, :], start=True, stop=True)
                        prevb = workp.tile([128, 128], mybir.dt.float32, tag="prevb")
                        nc.vector.tensor_tensor(
                            out=prevb[:], in0=x_own[:, t, :], in1=bbc_sb[:, l, :],
                            op=mybir.AluOpType.add)
                        t2 = workp.tile([128, 128], mybir.dt.float32, tag="t2")
                        nc.vector.tensor_scalar(
                            out=t2[:], in0=pso[:], scalar1=disT_sb[:, t:t + 1],
                            scalar2=None, op0=mybir.AluOpType.mult)
                        t3 = workp.tile([128, 128], mybir.dt.float32, tag="t3")
                        nc.vector.tensor_tensor(
                            out=t3[:], in0=t2[:], in1=prevb[:], op=mybir.AluOpType.add)
                        nc.scalar.activation(
                            out=x_own[:, t, :], in_=t3[:],
                            func=mybir.ActivationFunctionType.Relu)
                        if l < L - 1:
                            nc.scalar.activation(
                                out=xpr_all[:, t, :], in_=x_own[:, t, :],
                                func=mybir.ActivationFunctionType.Copy,
                                scale=disT_sb[:, t:t + 1])
                    if l < L - 1:
                        t0, t1 = tg[0], tg[-1] + 1
                        nc.sync.dma_start(
                            out=ag_ins[l].rearrange("(t p) f -> p t f", p=128)[:, t0:t1, :],
                            in_=xpr_all[:, t0:t1, :])
                if l < L - 1:
                    nc.gpsimd.collective_compute(
                        "AllGather",
                        mybir.AluOpType.bypass,
                        replica_groups=[list(range(NCORES))],
                        ins=[ag_ins[l].opt()],
                        outs=[tables[l + 1].opt()],
                    )
            nc.sync.dma_start(out=d_out.ap().rearrange("(t p) f -> p t f", p=128),
                              in_=x_own[:])

    nc.compile()
    return nc


def kernel(x, edge_index, W, b):
    global _compiled
    from concourse import bass_utils

    x = np.asarray(x, dtype=np.float32)
    W_np = np.asarray(W, dtype=np.float32)
    b_np = np.asarray(b, dtype=np.float32)
    ei = np.asarray(edge_index)

    meta, per_core = _preprocess(x, ei, W_np, b_np)
    globals()["_last_per_core"] = per_core
    if _compiled is None:
        _compiled = _build(meta)
    nc = _compiled
    res = bass_utils.run_bass_kernel_spmd(nc, per_core, core_ids=list(range(NCORES)))
    out = np.concatenate([res.results[c]["x_out"] for c in range(NCORES)], axis=0)
    return out[:N].astype(np.float32)


# revision 10
# speedup vs baseline: 2.4545x; 1.0080x over previous
"""Trainium2 Bass kernel for a 4-layer GCN (N=50000, D=128, E=1600000, 8 cores).

Hybrid aggregation (v3, group-major pipeline):
  - Nodes padded to 50176 = 392*128; each core owns 6272 nodes (49 tiles).
  - out = dis * ((sum_{e->v} x'[src_e]) @ W) + b + prev, x' = dis * x.
  - Src tiles split: DENSE = global tiles [0, M_DENSE) (same for all cores so
    the SPMD stream is identical; per-core B data differs), GATHER = the rest.
    Self-loops via an identity matmul from the SBUF-resident x' own slice.
  - 49 dst tiles -> 13 groups of <=4 (512 dst columns, one PSUM bank each),
    processed as a pipeline; per group:
      dense:  psum[fi,dst] += xall[:,s,:]^T @ B[s, g-slice]   (fp8 B streamed
              from HBM in 24-src-tile sub-blocks, N=512 moving operands)
      gather: dma_gather x' rows (bf16 edge-major, ~8ns/row Q7) + one-hot fp8
              S chunks: psum += msg_chunk^T @ S_chunk  (per 2-tile units)
      self:   psum[:, tl] += xpr_own_tile^T @ I
      close:  aggT bf16 <- psum; per tile: @W -> psum_out; epilogue on DVE/ACT
  - x' slices AllGather'd into the next layer's table; dense x' tiles bulk
    reloaded to SBUF each layer (no per-edge descriptors on the dense path).
"""

import sys

sys.path.insert(0, "/opt/trn_rl_repo")

import numpy as np
import ml_dtypes

N = 50000
D = 128
L = 4
E = 1600000
NCORES = 8
NPAD = 50176          # 392 * 128
NT = NPAD // 128      # 392 src tiles
NPC = NPAD // NCORES  # 6272
TPC = NPC // 128      # 49
M_DENSE = 192         # dense src tiles (global ids [0, M_DENSE))
DENSE_ROWS = M_DENSE * 128  # 24576; gathered span 25600 < 2^15
SBW = 24              # dense src tiles per B sub-block DMA
NSB = M_DENSE // SBW  # 8
GROUP = 4
NGROUPS = (TPC + GROUP - 1) // GROUP  # 13 (last group = 1 tile)
GW = 512              # psum group width (4*128)

_compiled = None


def _units_of_group(tg):
    return [tg[i:i + 2] for i in range(0, len(tg), 2)]


def _node_perm():
    """old node id -> rank-major-by-half table row.

    Part A (rows [0, 24576)): core c's tiles 0..23 at c*3072 + t*128 + p.
    Part B (rows [24576, 50176)): core c's tiles 24..48 at
    24576 + c*3200 + (t-24)*128 + p.
    """
    n = np.arange(NPAD, dtype=np.int64)
    c = n // NPC
    r = n % NPC
    t = r // 128
    p = r % 128
    return np.where(t < 24,
                    c * 3072 + t * 128 + p,
                    DENSE_ROWS + c * 3200 + (t - 24) * 128 + p)


def _preprocess(x, edge_index, W, b):
    fp8 = ml_dtypes.float8_e4m3
    src_old = edge_index[0].astype(np.int64)
    dst0 = edge_index[1].astype(np.int64)
    loops = np.arange(N, dtype=np.int64)
    deg = np.bincount(np.concatenate([dst0, loops]), minlength=N).astype(np.float32)
    dis = np.zeros(NPAD, np.float32)
    dis[:N] = 1.0 / np.sqrt(deg)

    perm = _node_perm()
    src0 = perm[src_old]
    core_of = dst0 // NPC
    is_dense = src0 < DENSE_ROWS

    # ---------------- dense path: B count blocks, group-major ----------------
    # BG[c] layout: [NGROUPS, NSB, 128(u), SBW(k), GW(dst col)]
    BG = np.zeros((NCORES, NGROUPS, NSB, 128, SBW, GW), np.uint8)
    de = np.where(is_dense)[0]
    d_core = core_of[de]
    st = src0[de] // 128
    d_sb = st // SBW
    d_k = st % SBW
    d_u = src0[de] % 128
    d_dloc = dst0[de] - d_core * NPC
    d_g = d_dloc // GW
    d_col = d_dloc % GW
    np.add.at(BG, (d_core, d_g, d_sb, d_u, d_k, d_col), 1)

    # ---------------- gather path ----------------
    ge = np.where(~is_dense)[0]
    gsrc, gdst = src0[ge], dst0[ge]
    g_core = core_of[ge]
    lt_of = (gdst % NPC) // 128
    dloc_of = gdst % 128
    key = g_core * TPC + lt_of
    order = np.argsort(key, kind="stable")
    src_s = gsrc[order]
    dloc_s = dloc_of[order]
    counts = np.bincount(key, minlength=NCORES * TPC).reshape(NCORES, TPC)
    KCH = np.maximum(np.ceil(counts.max(axis=0) / 128.0).astype(np.int64), 1)  # [TPC]

    tiles_of_group = [list(range(g * GROUP, min((g + 1) * GROUP, TPC)))
                      for g in range(NGROUPS)]
    TOTCH = int(KCH.sum())
    chunk_off = np.zeros(TPC, np.int64)
    pos = 0
    for tg in tiles_of_group:
        for t in tg:
            chunk_off[t] = pos
            pos += KCH[t]
    assert pos == TOTCH

    idx_all = np.zeros((NCORES, TOTCH * 128), np.int16)
    dlocs = np.full((NCORES, TOTCH * 128), -1, np.int16)
    starts = np.zeros(NCORES * TPC + 1, np.int64)
    np.cumsum(counts.reshape(-1), out=starts[1:])
    for c in range(NCORES):
        for t in range(TPC):
            k = c * TPC + t
            s0, s1 = starts[k], starts[k + 1]
            n = s1 - s0
            o = chunk_off[t] * 128
            idx_all[c, o:o + n] = (src_s[s0:s1] - DENSE_ROWS).astype(np.int16)
            dlocs[c, o:o + n] = dloc_s[s0:s1]

    smat = np.zeros((NCORES, TOTCH, 128, 128), fp8)
    ii = np.arange(TOTCH * 128)
    for c in range(NCORES):
        d = dlocs[c]
        m = d >= 0
        smat[c][ii[m] // 128, ii[m] % 128, d[m]] = 1.0

    idxw = np.zeros((NCORES, 128, TOTCH * 8), np.int16)
    for c in range(NCORES):
        w = idx_all[c].reshape(TOTCH * 8, 16).T
        for gme in range(8):
            idxw[c, gme * 16:(gme + 1) * 16, :] = w

    xpad = np.zeros((NPAD, D), np.float32)
    xpad[:N] = x
    x0p_node = (xpad * dis[:, None]).astype(ml_dtypes.bfloat16)
    x0p = np.empty_like(x0p_node)
    x0p[perm] = x0p_node  # table in rank-major-by-half layout
    disT = dis.reshape(NCORES, TPC, 128).transpose(0, 2, 1).copy()  # [c,128,TPC]
    b_bc = np.broadcast_to(b[None, :, :], (128, L, D)).astype(np.float32).copy()
    ident = np.eye(128, dtype=np.float32).astype(ml_dtypes.bfloat16)

    meta = dict(KCH=KCH, TOTCH=TOTCH, chunk_off=chunk_off,
                tiles_of_group=tiles_of_group)
    per_core = []
    for c in range(NCORES):
        x0p_own = np.ascontiguousarray(
            x0p_node[c * NPC:(c + 1) * NPC].reshape(TPC, 128, D).transpose(1, 0, 2))
        per_core.append(dict(
            x_own=np.ascontiguousarray(xpad[c * NPC:(c + 1) * NPC]),
            x0p=x0p,
            x0p_own=x0p_own,
            w=W.astype(ml_dtypes.bfloat16),
            b_bc=b_bc,
            disT=np.ascontiguousarray(disT[c]),
            ident=ident,
            idxs=np.ascontiguousarray(idxw[c]),
            smat=np.ascontiguousarray(smat[c].reshape(TOTCH, 128, 128)),
            bg=BG[c].astype(fp8),
        ))
    return meta, per_core


def _build(meta):
    from concourse import bacc, tile
    from concourse.bass import mybir

    KCH = meta["KCH"]
    TOTCH = meta["TOTCH"]
    chunk_off = meta["chunk_off"]
    tiles_of_group = meta["tiles_of_group"]
    MAXCH_U = max(int(sum(KCH[t] for t in u))
                  for tg in tiles_of_group for u in _units_of_group(tg))

    nc = bacc.Bacc("TRN2", target_bir_lowering=False, debug=False,
                   num_devices=NCORES)
    d_x_own = nc.dram_tensor("x_own", [NPC, D], mybir.dt.float32, kind="ExternalInput")
    d_x0p = nc.dram_tensor("x0p", [NPAD, D], mybir.dt.bfloat16, kind="ExternalInput")
    d_x0po = nc.dram_tensor("x0p_own", [128, TPC, D], mybir.dt.bfloat16, kind="ExternalInput")
    d_w = nc.dram_tensor("w", [L, D, D], mybir.dt.bfloat16, kind="ExternalInput")
    d_bbc = nc.dram_tensor("b_bc", [128, L, D], mybir.dt.float32, kind="ExternalInput")
    d_disT = nc.dram_tensor("disT", [128, TPC], mybir.dt.float32, kind="ExternalInput")
    d_ident = nc.dram_tensor("ident", [128, 128], mybir.dt.bfloat16, kind="ExternalInput")
    d_idxs = nc.dram_tensor("idxs", [128, TOTCH * 8], mybir.dt.int16, kind="ExternalInput")
    d_smat = nc.dram_tensor("smat", [TOTCH, 128, 128], mybir.dt.float8e4, kind="ExternalInput")
    d_bg = nc.dram_tensor("bg", [NGROUPS, NSB, 128, SBW, GW], mybir.dt.float8e4,
                          kind="ExternalInput")
    d_out = nc.dram_tensor("x_out", [NPC, D], mybir.dt.float32, kind="ExternalOutput")

    with tile.TileContext(nc) as tc:
        with (
            tc.tile_pool(name="const", bufs=1) as constp,
            tc.tile_pool(name="xallp", bufs=1) as xallp,
            tc.tile_pool(name="msg", bufs=3) as msgp,
            tc.tile_pool(name="sp", bufs=3) as sp_pool,
            tc.tile_pool(name="bblk", bufs=2) as bbp,
            tc.tile_pool(name="work", bufs=3) as workp,
            tc.tile_pool(name="aggt", bufs=2) as aggp,
            tc.tile_pool(name="pag", bufs=3, space="PSUM") as pagp,
            tc.tile_pool(name="pout", bufs=2, space="PSUM") as poutp,
            tc.tile_pool(name="dram", bufs=1, space="DRAM") as dramp,
        ):
            x_own = constp.tile([128, TPC, D], mybir.dt.float32, tag="x_own")
            nc.sync.dma_start(out=x_own[:], in_=d_x_own.ap().rearrange("(t p) f -> p t f", p=128))
            xpr_all = constp.tile([128, TPC, D], mybir.dt.bfloat16, tag="xpr_all")
            nc.sync.dma_start(out=xpr_all[:], in_=d_x0po.ap())
            w_sb = constp.tile([128, L, D], mybir.dt.bfloat16, tag="w_sb")
            nc.sync.dma_start(out=w_sb[:], in_=d_w.ap().rearrange("l k f -> k l f"))
            bbc_sb = constp.tile([128, L, D], mybir.dt.float32, tag="bbc")
            nc.sync.dma_start(out=bbc_sb[:], in_=d_bbc.ap())
            disT_sb = constp.tile([128, TPC], mybir.dt.float32, tag="disT")
            nc.sync.dma_start(out=disT_sb[:], in_=d_disT.ap())
            ident_sb = constp.tile([128, 128], mybir.dt.bfloat16, tag="ident")
            nc.sync.dma_start(out=ident_sb[:], in_=d_ident.ap())
            idx_sb = constp.tile([128, TOTCH * 8], mybir.dt.int16, tag="idx")
            nc.sync.dma_start(out=idx_sb[:], in_=d_idxs.ap())

            tables = [d_x0p.ap()]
            ag_ins_a = []
            ag_ins_b = []
            for l in range(1, L):
                tab_tile = dramp.tile([NPAD, D], mybir.dt.bfloat16, tag=f"tab{l}")
                agina_tile = dramp.tile([24 * 128, D], mybir.dt.bfloat16, tag=f"agina{l}")
                aginb_tile = dramp.tile([25 * 128, D], mybir.dt.bfloat16, tag=f"aginb{l}")
                tables.append(tab_tile[:])
                ag_ins_a.append(agina_tile[:])
                ag_ins_b.append(aginb_tile[:])

            GROUP_ORDER = list(range(6, NGROUPS)) + list(range(0, 6))
            for l in range(L):
                table = tables[l]
                tabG = table[DENSE_ROWS:NPAD, :]
                xall = xallp.tile([128, M_DENSE, D], mybir.dt.bfloat16,
                                  tag="xall", name=f"xall_{l}")
                nc.sync.dma_start(
                    out=xall[:],
                    in_=table[0:DENSE_ROWS, :].rearrange("(t p) f -> p t f", p=128))
                for g in GROUP_ORDER:
                    tg = tiles_of_group[g]
                    psum_g = pagp.tile([128, GW], mybir.dt.float32, tag="pag",
                                       name=f"pag_{l}_{g}")
                    # ---- dense stream ----
                    for sb in range(NSB):
                        bsub = bbp.tile([128, SBW, GW], mybir.dt.float8e4, tag="bsub",
                                        name=f"bsub_{l}_{g}_{sb}")
                        nc.sync.dma_start(out=bsub[:], in_=d_bg.ap()[g, sb])
                        for k in range(SBW):
                            nc.tensor.matmul(
                                psum_g[:],
                                lhsT=xall[:, sb * SBW + k, :],
                                rhs=bsub[:, k, :],
                                start=(sb == 0 and k == 0), stop=False)
                    # ---- gather stream (2-tile units) ----
                    for u in _units_of_group(tg):
                        nch = int(sum(KCH[t] for t in u))
                        goff = int(chunk_off[u[0]])
                        s_t = sp_pool.tile([128, MAXCH_U, 128], mybir.dt.float8e4,
                                           tag="s_t", name=f"s_t_{l}_{g}_{u[0]}")
                        nc.sync.dma_start(
                            out=s_t[:, 0:nch, :],
                            in_=d_smat.ap()[goff:goff + nch].rearrange("c p f -> p c f"))
                        msg = msgp.tile([128, MAXCH_U, D], mybir.dt.bfloat16,
                                        tag="msg", name=f"msg_{l}_{g}_{u[0]}")
                        nc.gpsimd.dma_gather(
                            out_ap=msg[:, 0:nch, :],
                            in_ap=tabG,
                            idxs_ap=idx_sb[:, goff * 8:(goff + nch) * 8],
                            num_idxs=nch * 128,
                            num_idxs_reg=nch * 128,
                            elem_size=D,
                            single_packet=False,
                        )
                        for t in u:
                            tl = t - tg[0]
                            oT = int(chunk_off[t] - goff)
                            for c in range(int(KCH[t])):
                                nc.tensor.matmul(
                                    psum_g[:, tl * 128:(tl + 1) * 128],
                                    lhsT=msg[:, oT + c, :],
                                    rhs=s_t[:, oT + c, :],
                                    start=False, stop=False)
                    # ---- self-loops, close, epilogue ----
                    for tl, t in enumerate(tg):
                        nc.tensor.matmul(
                            psum_g[:, tl * 128:(tl + 1) * 128],
                            lhsT=xpr_all[:, t, :],
                            rhs=ident_sb[:],
                            start=False, stop=(tl == len(tg) - 1))
                    wid = len(tg) * 128
                    aggT = aggp.tile([128, GW], mybir.dt.bfloat16, tag="aggT",
                                     name=f"aggT_{l}_{g}")
                    nc.scalar.copy(aggT[:, 0:wid], psum_g[:, 0:wid])
                    for tl, t in enumerate(tg):
                        pso = poutp.tile([128, 128], mybir.dt.float32, tag="pout",
                                         name=f"pout_{l}_{t}")
                        nc.tensor.matmul(pso[:], lhsT=aggT[:, tl * 128:(tl + 1) * 128],
                                         rhs=w_sb[:, l, :], start=True, stop=True)
                        prevb = workp.tile([128, 128], mybir.dt.float32, tag="prevb")
                        nc.vector.tensor_tensor(
                            out=prevb[:], in0=x_own[:, t, :], in1=bbc_sb[:, l, :],
                            op=mybir.AluOpType.add)
                        t2 = workp.tile([128, 128], mybir.dt.float32, tag="t2")
                        nc.vector.tensor_scalar(
                            out=t2[:], in0=pso[:], scalar1=disT_sb[:, t:t + 1],
                            scalar2=None, op0=mybir.AluOpType.mult)
                        t3 = workp.tile([128, 128], mybir.dt.float32, tag="t3")
                        nc.vector.tensor_tensor(
                            out=t3[:], in0=t2[:], in1=prevb[:], op=mybir.AluOpType.add)
                        nc.scalar.activation(
                            out=x_own[:, t, :], in_=t3[:],
                            func=mybir.ActivationFunctionType.Relu)
                        if l < L - 1:
                            nc.scalar.activation(
                                out=xpr_all[:, t, :], in_=x_own[:, t, :],
                                func=mybir.ActivationFunctionType.Copy,
                                scale=disT_sb[:, t:t + 1])
                    if l < L - 1:
                        t0, t1 = tg[0], tg[-1] + 1
                        if t0 >= 24:
                            nc.sync.dma_start(
                                out=ag_ins_b[l].rearrange("(t p) f -> p t f", p=128)
                                    [:, t0 - 24:t1 - 24, :],
                                in_=xpr_all[:, t0:t1, :])
                        else:
                            nc.sync.dma_start(
                                out=ag_ins_a[l].rearrange("(t p) f -> p t f", p=128)
                                    [:, t0:t1, :],
                                in_=xpr_all[:, t0:t1, :])
                    # fire AG_B as soon as the part-B dst groups (processed
                    # first) are done so next layer's gathers never wait;
                    # AG_A at layer end gates only the slack-rich dense path.
                    if l < L - 1 and g == NGROUPS - 1:
                        nc.gpsimd.collective_compute(
                            "AllGather",
                            mybir.AluOpType.bypass,
                            replica_groups=[list(range(NCORES))],
                            ins=[ag_ins_b[l].opt()],
                            outs=[tables[l + 1][DENSE_ROWS:NPAD, :].opt()],
                        )
                    if l < L - 1 and g == 5:
                        nc.gpsimd.collective_compute(
                            "AllGather",
                            mybir.AluOpType.bypass,
                            replica_groups=[list(range(NCORES))],
                            ins=[ag_ins_a[l].opt()],
                            outs=[tables[l + 1][0:DENSE_ROWS, :].opt()],
                        )
            nc.sync.dma_start(out=d_out.ap().rearrange("(t p) f -> p t f", p=128),
                              in_=x_own[:])

    nc.compile()
    return nc


def kernel(x, edge_index, W, b):
    global _compiled
    from concourse import bass_utils

    x = np.asarray(x, dtype=np.float32)
    W_np = np.asarray(W, dtype=np.float32)
    b_np = np.asarray(b, dtype=np.float32)
    ei = np.asarray(edge_index)

    meta, per_core = _preprocess(x, ei, W_np, b_np)
    globals()["_last_per_core"] = per_core
    if _compiled is None:
        _compiled = _build(meta)
    nc = _compiled
    res = bass_utils.run_bass_kernel_spmd(nc, per_core, core_ids=list(range(NCORES)))
    out = np.concatenate([res.results[c]["x_out"] for c in range(NCORES)], axis=0)
    return out[:N].astype(np.float32)
